# revision 27
# baseline (speedup 1.0000x reference)
"""LinearOffsetLayer Trainium2 kernel (8 NeuronCores, tensor-parallel on out_features).

Math:  A[o,i] = sum_d theta_d[d] * P_A[o,d,i] + theta0_A[o,i]
       b[o]   = theta_d @ P_b + theta0_b
       out    = input @ A.T + b                          # [4096, 1024]

Sharding: out_features (o) split 8 ways -> 128 o per core.  Each core gets its
P_A / theta0_A / P_b / theta0_b shard; input (pre-transposed on host to
[in_f, n]) and theta_d are replicated.  Each core computes out_T shard
[128, 4096]; host concatenates and transposes back.

v13 - all-fp8 P_A via host-side error-feedback quantization
(72.1 us cost model vs v10's 94.5 us; hw-verified rel err 1.784e-2
against the 2e-2 gate; inputs are deterministic, seed 0):

  The kernel is DMA-bandwidth-bound (HBM-per-NC ~358 GB/s); the P_A stream
  dominates.  v10 shipped P_A as 40 fp16 + 88 premultiplied-fp8 planes
  (1.3125 B/elem, 22 MB/core, quant err ~1e-2).  v12 ships ALL 128 planes
  as premultiplied fp8 (1 B/elem, 16 MB/core) using error feedback:
  planes are quantized sequentially in descending |theta| order, each
  plane's quantization residual carried into the next plane's values.
  The decode sum telescopes, cancelling every plane's rounding error
  except the last (smallest-|theta|) one:  A-shard rel err 4.3e-5 (30x
  better than v10 with 27% fewer bytes).  theta0_A's own fp8 rounding
  residual seeds the carry, so its error is absorbed too.  The fixed
  power-of-2 scale 2^6 (exact fp8 stationary value, so decode products
  are exact) keeps values mid-range; clip errors feed the carry; the
  host clamps away fp8 subnormals so PE flush behavior cannot matter.

  Einsum: ONE DoubleRow two-hot stream: pa8[p, ks, pc, i] holds rows
  o_a(pc) (ks=0) and o_a(pc)+8 (ks=1) of plane p; the stationary window
  holds 2^-6 at the matching hot columns.  128 DR
  matmuls x 512 out-cols accumulate A_off into 2 shared PSUM banks
  (~27 us PE incl. LDWEIGHTS, under the ~45 us P_A DMA stream).

  The freed error budget buys x bandwidth: the first 2560 tokens ride
  fp8, the rest fp16 (5.5 MB/core vs 8).  Output error is dominated by
  the fp8-token rows: host-sim 1.784e-2 == hw-measured (the host sim
  matches device arithmetic to 4 digits).  DMA total 22.7 MB/core.

  Schedule: the stationary window is a constant pattern (fixed scale
  2^6), built on-device with 3 Pool memsets -- no const DMA ahead of the
  pa8 stream, and warmup starts at ~0.4 us; t0aT + cst ride behind pa8;
  all x blocks are issued up front (they fit in SBUF) so the x stream
  runs uninterrupted; stores ride the Activation HWDGE queue (last two
  on SP after the final x load); the x stream ends with two 256-wide
  blocks to shorten the final matmul->evict->store chain; 24 warmup
  matmuls ramp the PE clock and 10 filler matmuls per pa-tile gap keep
  the HAM from re-throttling.

Per-core dataflow:
  1. einsum into PSUM rows: DR two-hot window stream accumulates
     A_off[o, i] into 2 shared PSUM banks; DVE evicts to a_sb.
  2. PE transpose per k-block, DVE adds theta0_A_T -> aT_sb [i, o] fp16.
  3. main matmul: out_T[:, nb] = sum_k aT_sb[k].T @ x_nb[k], PSUM k-inner
     accumulation, bias fused into the PSUM->SBUF eviction.
"""

from contextlib import ExitStack

import ml_dtypes
import numpy as np

import concourse.bacc as bacc
import concourse.mybir as mybir
import concourse.tile as tile
from concourse.bass_utils import run_bass_kernel_spmd
from concourse.masks import make_identity

P = 128          # partitions / d / per-core o-shard
IN_F = 1024
OUT_F = 1024
NTOK = 4096
NCORES = 8
KB = IN_F // P   # 8 k-blocks of the contraction dim
FD = 512         # PSUM bank free dim (f32 accumulators per partition)
NH = IN_F // FD  # 2 i-halves per o-row in the einsum
F32 = mybir.dt.float32
F16 = mybir.dt.float16
F8 = mybir.dt.float8e4
E4M3 = ml_dtypes.float8_e4m3
PAIR = 8         # two-hot pairs (o, o+PAIR) within each 16-o block

N8 = 2560        # leading tokens shipped fp8; rest fp16
X8_BLOCKS = [FD] * (N8 // FD)
X16_BLOCKS = [FD] * ((NTOK - N8) // FD - 1) + [256, 256]
assert N8 + sum(X16_BLOCKS) == NTOK

F8_MAX = 224.0           # TRN e4m3 max finite is 240 (inf at 256)
F8_MIN_NORMAL = 2.0 ** -6
Q_S = 64.0               # fixed premultiply scale 2^6 (the adaptive rms
                         # rule always chose it; clip errors are absorbed by
                         # the feedback carry, so a fixed scale is safe)
PA_BUFS = 3
X8_BUFS = 4
X16_BUFS = 4
WARMUP_MM = 24   # dummy PE matmuls to ramp the clock before the einsum
FILL_MM = 10     # keep-warm PE matmuls interleaved after each pa tile
CONSTS_ON_SP = True   # stream win8/t0a/cst ahead of pa8 on the SP queue
OUT_F16 = True   # ship out_T as fp16 (host casts back to f32)

_CACHE = {}


def _emit_body(nc, tc, ctx, d, pools, identity):
    (consts, x8_pool, x16_pool, pa_pool, asb_pool, ps_r, ps_o, outsb) = pools

    # The stationary window is a CONSTANT pattern (the quantizer uses the
    # fixed scale S=2^6; decode weight 2^-6 at the two hot diagonals), so it
    # is built on-device with 3 Pool memsets -- no DMA, and the PE warmup
    # can start at ~0.4 us instead of waiting for a const load.
    win8_sb = consts.tile([P, 2, 2 * P], F8, name="win8_sb")
    nc.gpsimd.memset(win8_sb[:], 0.0)
    nc.gpsimd.memset(win8_sb[:, 0, P - 1:P], 1.0 / Q_S)
    nc.gpsimd.memset(win8_sb[:, 1, P - 1 + PAIR:P + PAIR], 1.0 / Q_S)

    # t0aT [P, KB, P] packed as [P, 4, 2P] (k-block k at slab k//2, half k%2)
    t8_sb = consts.tile([P, 4, 2 * P], F8, name="t8_sb")

    def t0a_slice(k):
        return t8_sb[:, k // 2, (k % 2) * P:(k % 2) * P + P]

    cst_sb = consts.tile([P, 2 + P], F16, name="cst_sb")
    th_sb = cst_sb[:, 0:1]
    t0b_sb = cst_sb[:, 1:2]
    pb_sb = cst_sb[:, 2:2 + P]
    b_sb = consts.tile([P, 1], F32, name="b_sb")

    # PE warmup: the tensor engine's clock ramps over ~3.4us of continuous
    # work; dummy matmuls on the (device-built) window const bring it
    # to full pstate while the first P_A tiles are still in flight.
    for _ in range(WARMUP_MM):
        wup = ps_o.tile([P, 2 * P], F32, name="wup", tag="po")
        nc.tensor.matmul(wup[:], lhsT=win8_sb[:, 0, 0:P], rhs=win8_sb[:, 0, :],
                         start=True, stop=True)

    # einsum: A_off[o, i] accumulated two-hot-row-at-a-time in 2 PSUM banks.
    ablk = [ps_r.tile([P, FD], F32, name=f"ablk{h}", tag="ablk")
            for h in range(NH)]
    for t in range(P // 16):          # 16 o-rows (8 two-hot pairs) per round
        pa_t = pa_pool.tile([P, 2, PAIR, IN_F], F8, name="pa_t")
        nc.sync.dma_start(pa_t[:], d["pa8"][:, :, t * PAIR:(t + 1) * PAIR, :])
        for r in range(PAIR):
            oa = t * 16 + r           # two-hot: rows oa and oa+PAIR
            for h in range(NH):
                nc.tensor.matmul(
                    ablk[h][:, :],
                    lhsT=win8_sb[:, :, P - 1 - oa:2 * P - 1 - oa],
                    rhs=pa_t[:, :, r, h * FD:(h + 1) * FD],
                    start=(oa == 0), stop=(t == P // 16 - 1 and r == PAIR - 1
                                           and h == NH - 1),
                    perf_mode=mybir.MatmulPerfMode.DoubleRow,
                    skip_group_check=True)
        # keep-warm fill: the einsum consumes a pa tile faster than DMA
        # supplies the next one; dependency-free matmuls on the window
        # const fill the PE-idle gap so the clock ramp (HAM) never drops.
        if t < P // 16 - 1:
            for _ in range(FILL_MM):
                wup = ps_o.tile([P, 2 * P], F32, name="wup", tag="po")
                nc.tensor.matmul(wup[:], lhsT=win8_sb[:, 0, 0:P],
                                 rhs=win8_sb[:, 0, :], start=True, stop=True)

    # t0aT + cst ride behind the pa8 stream (needed only now)
    nc.sync.dma_start(t8_sb[:], d["t8"][:, :, :])
    nc.sync.dma_start(cst_sb[:], d["cst"][:, :])

    # bias: b = P_b.T @ theta + theta0_b     [o, 1]
    bp = ps_o.tile([P, 1], F32, name="bp", tag="po")
    nc.tensor.matmul(bp[:], lhsT=pb_sb, rhs=th_sb, start=True, stop=True)
    nc.vector.tensor_add(b_sb[:], bp[:], t0b_sb)

    a_sb = asb_pool.tile([P, IN_F], F32, name="a_sb")
    for h in range(NH):
        nc.vector.tensor_copy(a_sb[:, h * FD:(h + 1) * FD], ablk[h][:, :])

    # transpose a_sb [o,i] -> aT_sb [i,o] via PE; fold in theta0_A_T
    aT_sb = asb_pool.tile([P, IN_F], F16, name="aT_sb")
    for k in range(KB):
        pt = ps_o.tile([P, P], F32, name="pt", tag="po")
        nc.tensor.transpose(pt[:], a_sb[:, k * P:(k + 1) * P], identity[:])
        nc.vector.tensor_add(
            aT_sb[:, k * P:(k + 1) * P], pt[:], t0a_slice(k))

    # main matmul: out_T[:, nb] = sum_k aT_sb[k].T @ x_nb[k] ; + b.
    # x streams nb-major here, AFTER the P_A stream in DMA-queue order:
    # fp8 token blocks first, then fp16.
    out_dt = F16 if OUT_F16 else F32
    def _offsets(blocks):
        offs, acc = [], 0
        for w in blocks:
            offs.append(acc)
            acc += w
        return offs

    nblocks = [(w, F8, "xT8", n0o) for w, n0o in
               zip(X8_BLOCKS, _offsets(X8_BLOCKS))]
    nblocks += [(w, F16, "xT16", n0o) for w, n0o in
                zip(X16_BLOCKS, _offsets(X16_BLOCKS))]
    # all x blocks fit in SBUF at once, so issue every x load up front:
    # the x stream runs uninterrupted on the SP queue, and the stores
    # (emitted after, same queue) drain behind it during the tail without
    # ever delaying an x arrival.
    xtiles = []
    for w, xdt, xname, n0off in nblocks:
        pool = x8_pool if xdt == F8 else x16_pool
        xnb = pool.tile([P, KB, w], xdt, name=f"xnb{xdt}")
        nc.sync.dma_start(
            xnb[:],
            d[xname][:, n0off:n0off + w].rearrange("(k p) n -> p k n", p=P))
        xtiles.append(xnb)
    n0 = 0
    nb_total = len(nblocks)
    for bi, ((w, xdt, xname, n0off), xnb) in enumerate(zip(nblocks, xtiles)):
        po = ps_o.tile([P, w], F32, name="po", tag="po")
        for k in range(KB):
            nc.tensor.matmul(
                po[:],
                lhsT=aT_sb[:, k * P:(k + 1) * P],
                rhs=xnb[:, k, :],
                start=(k == 0), stop=(k == KB - 1))
        ot = outsb.tile([P, w], out_dt, name="ot")
        nc.vector.tensor_scalar_add(ot[:], po[:], b_sb[:, 0:1])
        # stores ride the Activation HWDGE queue so a store waiting on its
        # eviction can't head-of-line-block anything on SP's queue; the
        # last two take SP's (lower-latency, by-then-drained) DGE instead.
        eng = nc.sync if bi >= nb_total - 2 else nc.scalar
        eng.dma_start(d["out"][:, n0:n0 + w], ot[:])
        n0 += w


def _build(reps=1):
    nc = bacc.Bacc("TRN2", target_bir_lowering=False, debug=False,
                   num_devices=NCORES)

    d = {
        "xT8": nc.dram_tensor("xT8", [IN_F, N8], F8, kind="ExternalInput"),
        "xT16": nc.dram_tensor("xT16", [IN_F, NTOK - N8], F16,
                               kind="ExternalInput"),
        # [p, ks, pc, i]: plane p, row o_a(pc) + PAIR*ks,
        # where o_a(pc) = 16*(pc//8) + pc%8
        "pa8": nc.dram_tensor("pa8", [P, 2, P // 2, IN_F], F8,
                              kind="ExternalInput"),
        # t0aT [i_loc, k, o] packed as [P, 4, 2P]
        "t8": nc.dram_tensor("t8", [P, 4, 2 * P], F8, kind="ExternalInput"),
        # packed consts: [:,0]=theta_d, [:,1]=theta0_b shard, [:,2:]=P_b shard
        "cst": nc.dram_tensor("cst", [P, 2 + P], F16, kind="ExternalInput"),
        "out": nc.dram_tensor("out", [P, NTOK], F16 if OUT_F16 else F32,
                              kind="ExternalOutput"),
    }

    with tile.TileContext(nc) as tc:
        with ExitStack() as ctx:
            pools = (
                ctx.enter_context(tc.tile_pool(name="consts", bufs=2)),
                ctx.enter_context(tc.tile_pool(name="x8", bufs=X8_BUFS)),
                ctx.enter_context(tc.tile_pool(name="x16", bufs=X16_BUFS)),
                ctx.enter_context(tc.tile_pool(name="pa", bufs=PA_BUFS)),
                ctx.enter_context(tc.tile_pool(name="asb", bufs=2)),
                ctx.enter_context(tc.tile_pool(name="ps_r", bufs=2,
                                               space="PSUM")),
                ctx.enter_context(tc.tile_pool(name="ps_o", bufs=3,
                                               space="PSUM")),
                ctx.enter_context(tc.tile_pool(name="outsb", bufs=4)),
            )
            const_pool = pools[0]
            identity = const_pool.tile([P, P], F32, name="identity")
            make_identity(nc, identity)
            for _ in range(reps):
                _emit_body(nc, tc, ctx, d, pools, identity)

    nc.compile()
    return nc


def _f8_quant(v):
    """f32 -> e4m3 (RNE), clipped to +-224, subnormals clamped to
    {0, +-2^-6} so PE subnormal-flush behavior cannot matter."""
    q = np.clip(v, -F8_MAX, F8_MAX).astype(E4M3)
    qf = q.astype(np.float32)
    aq = np.abs(qf)
    sub = (aq > 0) & (aq < F8_MIN_NORMAL)
    if sub.any():
        fixed = np.where(np.abs(v) >= F8_MIN_NORMAL / 2,
                         np.sign(v).astype(np.float32) * F8_MIN_NORMAL,
                         np.float32(0.0))
        q = np.where(sub, fixed.astype(E4M3), q)
    return q


def _in_maps(inputs):
    x = np.asarray(inputs["input"], dtype=np.float32)
    theta_d = np.asarray(inputs["theta_d"], dtype=np.float32)
    theta0_A = np.asarray(inputs["theta0_A"], dtype=np.float32)
    P_A = np.asarray(inputs["P_A"], dtype=np.float32)
    theta0_b = np.asarray(inputs["theta0_b"], dtype=np.float32)
    P_b = np.asarray(inputs["P_b"], dtype=np.float32)

    order = np.argsort(-np.abs(theta_d), kind="stable")

    xT = np.ascontiguousarray(x.T)                        # [in_f, n]
    xT8 = _f8_quant(xT[:, :N8])
    xT16 = np.ascontiguousarray(xT[:, N8:].astype(np.float16))
    # t0a host layout [i_loc, k, o]: t0a[p, k, o] = theta0_A.T[k*128+p, o]
    t0aT_q = _f8_quant(theta0_A.T).reshape(KB, P, OUT_F).transpose(1, 0, 2)

    # two-hot pair-column order: pc = 8t + r  ->  o_a = 16t + r
    o_a = (16 * (np.arange(P // 2) // PAIR) + np.arange(P // 2) % PAIR)

    maps = []
    for c in range(NCORES):
        o0 = c * P
        cst = np.empty((P, 2 + P), np.float16)
        cst[:, 0] = theta_d
        cst[:, 1] = theta0_b[o0:o0 + P]
        cst[:, 2:] = P_b[:, o0:o0 + P]
        pa_sh = P_A[o0:o0 + P]                               # [o, d, i]
        t0a_q = _f8_quant(theta0_A[o0:o0 + P])               # [o, i]

        # error-feedback quantization: descending |theta| plane order;
        # theta0_A's rounding residual seeds the carry.  Fixed scale Q_S:
        # the decode weight in the device-built window is exactly 1/Q_S.
        carry = theta0_A[o0:o0 + P] - t0a_q.astype(np.float32)
        pa8 = np.zeros((P, 2, P // 2, IN_F), E4M3)
        S = np.float32(Q_S)
        for dd in order:
            v = theta_d[dd] * pa_sh[:, dd, :] + carry        # [o, i]
            q = _f8_quant(v * S)                             # [o, i] e4m3
            carry = v - q.astype(np.float32) / S
            pa8[dd, 0] = q[o_a]                              # rows o_a
            pa8[dd, 1] = q[o_a + PAIR]                       # rows o_a+8
        t8 = np.ascontiguousarray(
            t0aT_q[:, :, o0:o0 + P].reshape(P, 4, 2 * P))

        maps.append({
            "xT8": xT8,
            "xT16": xT16,
            "pa8": pa8,
            "t8": t8,
            "cst": cst,
        })
    return maps


def run(inputs, trace=False):
    """Returns (output [4096,1024] f32, exec_time_ns or None)."""
    if "nc" not in _CACHE:
        _CACHE["nc"] = _build()
    nc = _CACHE["nc"]
    res = run_bass_kernel_spmd(nc, _in_maps(inputs),
                               core_ids=list(range(NCORES)), trace=trace)
    shards = [res.results[c]["out"] for c in range(NCORES)]   # [128, 4096] each
    outT = np.concatenate(shards, axis=0)                     # [out_f, n]
    return np.ascontiguousarray(outT.T.astype(np.float32)), res.exec_time_ns


def kernel(**inputs):
    out, _ = run(inputs, trace=False)
    return out


# revision 28
# speedup vs baseline: 1.0051x; 1.0051x over previous
"""LinearOffsetLayer Trainium2 kernel (8 NeuronCores, tensor-parallel on out_features).

Math:  A[o,i] = sum_d theta_d[d] * P_A[o,d,i] + theta0_A[o,i]
       b[o]   = theta_d @ P_b + theta0_b
       out    = input @ A.T + b                          # [4096, 1024]

Sharding: out_features (o) split 8 ways -> 128 o per core.  Each core gets its
P_A / theta0_A / P_b / theta0_b shard; input (pre-transposed on host to
[in_f, n]) and theta_d are replicated.  Each core computes out_T shard
[128, 4096]; host concatenates and transposes back.

v13 - all-fp8 P_A via host-side error-feedback quantization
(72.1 us cost model vs v10's 94.5 us; hw-verified rel err 1.784e-2
against the 2e-2 gate; inputs are deterministic, seed 0):

  The kernel is DMA-bandwidth-bound (HBM-per-NC ~358 GB/s); the P_A stream
  dominates.  v10 shipped P_A as 40 fp16 + 88 premultiplied-fp8 planes
  (1.3125 B/elem, 22 MB/core, quant err ~1e-2).  v12 ships ALL 128 planes
  as premultiplied fp8 (1 B/elem, 16 MB/core) using error feedback:
  planes are quantized sequentially in descending |theta| order, each
  plane's quantization residual carried into the next plane's values.
  The decode sum telescopes, cancelling every plane's rounding error
  except the last (smallest-|theta|) one:  A-shard rel err 4.3e-5 (30x
  better than v10 with 27% fewer bytes).  theta0_A's own fp8 rounding
  residual seeds the carry, so its error is absorbed too.  The fixed
  power-of-2 scale 2^6 (exact fp8 stationary value, so decode products
  are exact) keeps values mid-range; clip errors feed the carry; the
  host clamps away fp8 subnormals so PE flush behavior cannot matter.

  Einsum: ONE DoubleRow two-hot stream: pa8[p, ks, pc, i] holds rows
  o_a(pc) (ks=0) and o_a(pc)+8 (ks=1) of plane p; the stationary window
  holds 2^-6 at the matching hot columns.  128 DR
  matmuls x 512 out-cols accumulate A_off into 2 shared PSUM banks
  (~27 us PE incl. LDWEIGHTS, under the ~45 us P_A DMA stream).

  The freed error budget buys x bandwidth: the first 2560 tokens ride
  fp8, the rest fp16 (5.5 MB/core vs 8).  Output error is dominated by
  the fp8-token rows: host-sim 1.784e-2 == hw-measured (the host sim
  matches device arithmetic to 4 digits).  DMA total 22.7 MB/core.

  Schedule: the stationary window is a constant pattern (fixed scale
  2^6), built on-device with 3 Pool memsets -- no const DMA ahead of the
  pa8 stream, and warmup starts at ~0.4 us; t0aT + cst ride behind pa8;
  all x blocks are issued up front (they fit in SBUF) so the x stream
  runs uninterrupted; stores ride the Activation HWDGE queue (last two
  on SP after the final x load); the x stream ends with two 256-wide
  blocks to shorten the final matmul->evict->store chain; 24 warmup
  matmuls ramp the PE clock and 10 filler matmuls per pa-tile gap keep
  the HAM from re-throttling.

Per-core dataflow:
  1. einsum into PSUM rows: DR two-hot window stream accumulates
     A_off[o, i] into 2 shared PSUM banks; DVE evicts to a_sb.
  2. PE transpose per k-block, DVE adds theta0_A_T -> aT_sb [i, o] fp16.
  3. main matmul: out_T[:, nb] = sum_k aT_sb[k].T @ x_nb[k], PSUM k-inner
     accumulation, bias fused into the PSUM->SBUF eviction.
"""

from contextlib import ExitStack

import ml_dtypes
import numpy as np

import concourse.bacc as bacc
import concourse.mybir as mybir
import concourse.tile as tile
from concourse.bass_utils import run_bass_kernel_spmd
from concourse.masks import make_identity

P = 128          # partitions / d / per-core o-shard
IN_F = 1024
OUT_F = 1024
NTOK = 4096
NCORES = 8
KB = IN_F // P   # 8 k-blocks of the contraction dim
FD = 512         # PSUM bank free dim (f32 accumulators per partition)
NH = IN_F // FD  # 2 i-halves per o-row in the einsum
F32 = mybir.dt.float32
F16 = mybir.dt.float16
F8 = mybir.dt.float8e4
E4M3 = ml_dtypes.float8_e4m3
PAIR = 8         # two-hot pairs (o, o+PAIR) within each 16-o block

N8 = 2560        # leading tokens shipped fp8; rest fp16
X8_BLOCKS = [FD] * (N8 // FD)
X16_BLOCKS = [FD] * ((NTOK - N8) // FD - 1) + [256, 256]
assert N8 + sum(X16_BLOCKS) == NTOK

F8_MAX = 224.0           # TRN e4m3 max finite is 240 (inf at 256)
F8_MIN_NORMAL = 2.0 ** -6
Q_S = 64.0               # fixed premultiply scale 2^6 (the adaptive rms
                         # rule always chose it; clip errors are absorbed by
                         # the feedback carry, so a fixed scale is safe)
PA_BUFS = 3
X8_BUFS = 4
X16_BUFS = 4
WARMUP_MM = 24   # dummy PE matmuls to ramp the clock before the einsum
FILL_MM = 10     # keep-warm PE matmuls interleaved after each pa tile
CONSTS_ON_SP = True   # stream win8/t0a/cst ahead of pa8 on the SP queue
OUT_F16 = True   # ship out_T as fp16 (host casts back to f32)

_CACHE = {}


def _emit_body(nc, tc, ctx, d, pools, identity):
    (consts, x8_pool, x16_pool, pa_pool, asb_pool, ps_r, ps_o, outsb) = pools

    # The stationary window is a CONSTANT pattern (the quantizer uses the
    # fixed scale S=2^6; decode weight 2^-6 at the two hot diagonals), so it
    # is built on-device with 3 Pool memsets -- no DMA, and the PE warmup
    # can start at ~0.4 us instead of waiting for a const load.
    win8_sb = consts.tile([P, 2, 2 * P], F8, name="win8_sb")
    nc.gpsimd.memset(win8_sb[:], 0.0)
    nc.gpsimd.memset(win8_sb[:, 0, P - 1:P], 1.0 / Q_S)
    nc.gpsimd.memset(win8_sb[:, 1, P - 1 + PAIR:P + PAIR], 1.0 / Q_S)

    cst_sb = consts.tile([P, 2 + P], F16, name="cst_sb")
    th_sb = cst_sb[:, 0:1]
    t0b_sb = cst_sb[:, 1:2]
    pb_sb = cst_sb[:, 2:2 + P]
    b_sb = consts.tile([P, 1], F32, name="b_sb")

    # PE warmup: the tensor engine's clock ramps over ~3.4us of continuous
    # work; dummy matmuls on the (device-built) window const bring it
    # to full pstate while the first P_A tiles are still in flight.
    for _ in range(WARMUP_MM):
        wup = ps_o.tile([P, 2 * P], F32, name="wup", tag="po")
        nc.tensor.matmul(wup[:], lhsT=win8_sb[:, 0, 0:P], rhs=win8_sb[:, 0, :],
                         start=True, stop=True)

    # einsum: A_off[o, i] accumulated two-hot-row-at-a-time in 2 PSUM banks.
    ablk = [ps_r.tile([P, FD], F32, name=f"ablk{h}", tag="ablk")
            for h in range(NH)]
    for t in range(P // 16):          # 16 o-rows (8 two-hot pairs) per round
        pa_t = pa_pool.tile([P, 2, PAIR, IN_F], F8, name="pa_t")
        nc.sync.dma_start(pa_t[:], d["pa8"][:, :, t * PAIR:(t + 1) * PAIR, :])
        for r in range(PAIR):
            oa = t * 16 + r           # two-hot: rows oa and oa+PAIR
            for h in range(NH):
                nc.tensor.matmul(
                    ablk[h][:, :],
                    lhsT=win8_sb[:, :, P - 1 - oa:2 * P - 1 - oa],
                    rhs=pa_t[:, :, r, h * FD:(h + 1) * FD],
                    start=(oa == 0), stop=(t == P // 16 - 1 and r == PAIR - 1
                                           and h == NH - 1),
                    perf_mode=mybir.MatmulPerfMode.DoubleRow,
                    skip_group_check=True)
        # keep-warm fill: the einsum consumes a pa tile faster than DMA
        # supplies the next one; dependency-free matmuls on the window
        # const fill the PE-idle gap so the clock ramp (HAM) never drops.
        if t < P // 16 - 1:
            for _ in range(FILL_MM):
                wup = ps_o.tile([P, 2 * P], F32, name="wup", tag="po")
                nc.tensor.matmul(wup[:], lhsT=win8_sb[:, 0, 0:P],
                                 rhs=win8_sb[:, 0, :], start=True, stop=True)

    # cst rides behind the pa8 stream (needed only now, for the bias)
    nc.sync.dma_start(cst_sb[:], d["cst"][:, :])

    # bias: b = P_b.T @ theta + theta0_b     [o, 1]
    bp = ps_o.tile([P, 1], F32, name="bp", tag="po")
    nc.tensor.matmul(bp[:], lhsT=pb_sb, rhs=th_sb, start=True, stop=True)
    nc.vector.tensor_add(b_sb[:], bp[:], t0b_sb)

    a_sb = asb_pool.tile([P, IN_F], F32, name="a_sb")
    for h in range(NH):
        nc.vector.tensor_copy(a_sb[:, h * FD:(h + 1) * FD], ablk[h][:, :])

    # transpose a_sb [o,i] -> aT_sb [i,o] via PE (theta0_A already rides
    # the einsum planes via the feedback carry seed)
    aT_sb = asb_pool.tile([P, IN_F], F16, name="aT_sb")
    for k in range(KB):
        pt = ps_o.tile([P, P], F32, name="pt", tag="po")
        nc.tensor.transpose(pt[:], a_sb[:, k * P:(k + 1) * P], identity[:])
        nc.vector.tensor_copy(aT_sb[:, k * P:(k + 1) * P], pt[:])

    # main matmul: out_T[:, nb] = sum_k aT_sb[k].T @ x_nb[k] ; + b.
    # x streams nb-major here, AFTER the P_A stream in DMA-queue order:
    # fp8 token blocks first, then fp16.
    out_dt = F16 if OUT_F16 else F32
    def _offsets(blocks):
        offs, acc = [], 0
        for w in blocks:
            offs.append(acc)
            acc += w
        return offs

    nblocks = [(w, F8, "xT8", n0o) for w, n0o in
               zip(X8_BLOCKS, _offsets(X8_BLOCKS))]
    nblocks += [(w, F16, "xT16", n0o) for w, n0o in
                zip(X16_BLOCKS, _offsets(X16_BLOCKS))]
    # all x blocks fit in SBUF at once, so issue every x load up front:
    # the x stream runs uninterrupted on the SP queue, and the stores
    # (emitted after, same queue) drain behind it during the tail without
    # ever delaying an x arrival.
    xtiles = []
    for w, xdt, xname, n0off in nblocks:
        pool = x8_pool if xdt == F8 else x16_pool
        xnb = pool.tile([P, KB, w], xdt, name=f"xnb{xdt}")
        nc.sync.dma_start(
            xnb[:],
            d[xname][:, n0off:n0off + w].rearrange("(k p) n -> p k n", p=P))
        xtiles.append(xnb)
    n0 = 0
    nb_total = len(nblocks)
    for bi, ((w, xdt, xname, n0off), xnb) in enumerate(zip(nblocks, xtiles)):
        po = ps_o.tile([P, w], F32, name="po", tag="po")
        for k in range(KB):
            nc.tensor.matmul(
                po[:],
                lhsT=aT_sb[:, k * P:(k + 1) * P],
                rhs=xnb[:, k, :],
                start=(k == 0), stop=(k == KB - 1))
        ot = outsb.tile([P, w], out_dt, name="ot")
        nc.vector.tensor_scalar_add(ot[:], po[:], b_sb[:, 0:1])
        # stores ride the Activation HWDGE queue so a store waiting on its
        # eviction can't head-of-line-block anything on SP's queue; the
        # last two take SP's (lower-latency, by-then-drained) DGE instead.
        eng = nc.sync if bi >= nb_total - 2 else nc.scalar
        eng.dma_start(d["out"][:, n0:n0 + w], ot[:])
        n0 += w


def _build(reps=1):
    nc = bacc.Bacc("TRN2", target_bir_lowering=False, debug=False,
                   num_devices=NCORES)

    d = {
        "xT8": nc.dram_tensor("xT8", [IN_F, N8], F8, kind="ExternalInput"),
        "xT16": nc.dram_tensor("xT16", [IN_F, NTOK - N8], F16,
                               kind="ExternalInput"),
        # [p, ks, pc, i]: plane p, row o_a(pc) + PAIR*ks,
        # where o_a(pc) = 16*(pc//8) + pc%8
        "pa8": nc.dram_tensor("pa8", [P, 2, P // 2, IN_F], F8,
                              kind="ExternalInput"),
        # packed consts: [:,0]=theta_d, [:,1]=theta0_b shard, [:,2:]=P_b shard
        "cst": nc.dram_tensor("cst", [P, 2 + P], F16, kind="ExternalInput"),
        "out": nc.dram_tensor("out", [P, NTOK], F16 if OUT_F16 else F32,
                              kind="ExternalOutput"),
    }

    with tile.TileContext(nc) as tc:
        with ExitStack() as ctx:
            pools = (
                ctx.enter_context(tc.tile_pool(name="consts", bufs=2)),
                ctx.enter_context(tc.tile_pool(name="x8", bufs=X8_BUFS)),
                ctx.enter_context(tc.tile_pool(name="x16", bufs=X16_BUFS)),
                ctx.enter_context(tc.tile_pool(name="pa", bufs=PA_BUFS)),
                ctx.enter_context(tc.tile_pool(name="asb", bufs=2)),
                ctx.enter_context(tc.tile_pool(name="ps_r", bufs=2,
                                               space="PSUM")),
                ctx.enter_context(tc.tile_pool(name="ps_o", bufs=3,
                                               space="PSUM")),
                ctx.enter_context(tc.tile_pool(name="outsb", bufs=4)),
            )
            const_pool = pools[0]
            identity = const_pool.tile([P, P], F32, name="identity")
            make_identity(nc, identity)
            for _ in range(reps):
                _emit_body(nc, tc, ctx, d, pools, identity)

    nc.compile()
    return nc


def _f8_quant(v):
    """f32 -> e4m3 (RNE), clipped to +-224, subnormals clamped to
    {0, +-2^-6} so PE subnormal-flush behavior cannot matter."""
    q = np.clip(v, -F8_MAX, F8_MAX).astype(E4M3)
    qf = q.astype(np.float32)
    aq = np.abs(qf)
    sub = (aq > 0) & (aq < F8_MIN_NORMAL)
    if sub.any():
        fixed = np.where(np.abs(v) >= F8_MIN_NORMAL / 2,
                         np.sign(v).astype(np.float32) * F8_MIN_NORMAL,
                         np.float32(0.0))
        q = np.where(sub, fixed.astype(E4M3), q)
    return q


def _in_maps(inputs):
    x = np.asarray(inputs["input"], dtype=np.float32)
    theta_d = np.asarray(inputs["theta_d"], dtype=np.float32)
    theta0_A = np.asarray(inputs["theta0_A"], dtype=np.float32)
    P_A = np.asarray(inputs["P_A"], dtype=np.float32)
    theta0_b = np.asarray(inputs["theta0_b"], dtype=np.float32)
    P_b = np.asarray(inputs["P_b"], dtype=np.float32)

    order = np.argsort(-np.abs(theta_d), kind="stable")

    xT = np.ascontiguousarray(x.T)                        # [in_f, n]
    xT8 = _f8_quant(xT[:, :N8])
    xT16 = np.ascontiguousarray(xT[:, N8:].astype(np.float16))

    # two-hot pair-column order: pc = 8t + r  ->  o_a = 16t + r
    o_a = (16 * (np.arange(P // 2) // PAIR) + np.arange(P // 2) % PAIR)

    maps = []
    for c in range(NCORES):
        o0 = c * P
        cst = np.empty((P, 2 + P), np.float16)
        cst[:, 0] = theta_d
        cst[:, 1] = theta0_b[o0:o0 + P]
        cst[:, 2:] = P_b[:, o0:o0 + P]
        pa_sh = P_A[o0:o0 + P]                               # [o, d, i]

        # error-feedback quantization: descending |theta| plane order;
        # ALL of theta0_A seeds the carry, so it rides the einsum planes
        # (absorbed by the first quantized plane, residuals telescoping)
        # and needs no separate tensor or device add.  Fixed scale Q_S:
        # the decode weight in the device-built window is exactly 1/Q_S.
        carry = theta0_A[o0:o0 + P].copy()
        pa8 = np.zeros((P, 2, P // 2, IN_F), E4M3)
        S = np.float32(Q_S)
        for dd in order:
            v = theta_d[dd] * pa_sh[:, dd, :] + carry        # [o, i]
            q = _f8_quant(v * S)                             # [o, i] e4m3
            carry = v - q.astype(np.float32) / S
            pa8[dd, 0] = q[o_a]                              # rows o_a
            pa8[dd, 1] = q[o_a + PAIR]                       # rows o_a+8
        maps.append({
            "xT8": xT8,
            "xT16": xT16,
            "pa8": pa8,
            "cst": cst,
        })
    return maps


def run(inputs, trace=False):
    """Returns (output [4096,1024] f32, exec_time_ns or None)."""
    if "nc" not in _CACHE:
        _CACHE["nc"] = _build()
    nc = _CACHE["nc"]
    res = run_bass_kernel_spmd(nc, _in_maps(inputs),
                               core_ids=list(range(NCORES)), trace=trace)
    shards = [res.results[c]["out"] for c in range(NCORES)]   # [128, 4096] each
    outT = np.concatenate(shards, axis=0)                     # [out_f, n]
    return np.ascontiguousarray(outT.T.astype(np.float32)), res.exec_time_ns


def kernel(**inputs):
    out, _ = run(inputs, trace=False)
    return out
